# revision 1
# baseline (speedup 1.0000x reference)
"""Multi-head attention kernel for Trainium2, 8 NeuronCores.

Problem (NHEAD=8, T=S=1024, B=8, A=512, hd=64):
  q = queries.reshape(T, B*NH, hd); k = keys.reshape(S, B*NH, hd)
  w = softmax(mask(q @ k^T / sqrt(hd)))      per n = b*NH + h, mask = attn_mask[n % NH]
  out = (w @ k).reshape(T, B, A)             (keys double as values)

Sharding: head-parallel. Core c owns head h=c for all 8 batches; every
problem on core c uses the single mask slice attn_mask[c] (n % 8 == h).

Per-core dataflow (all matmul operands bf16, f32 PSUM accumulation;
verified 1.8e-3 rel-L2 vs the f32 reference):
  - qT/kT [h, t] layouts prepared on host; two batches per 128-partition
    tile -> mm1 runs two batches concurrently via PE row tiling (K=64
    each, tile_position (0,0)/(64,0)).
  - mm1: scoresT[s_tile, t] = kT.T @ qT into PSUM [128, 2048] (batch pair).
  - ACT: p = exp(scoresT * 1/8) PSUM->SBUF bf16 (no max subtraction needed:
    |scores/8| <= ~6).
  - DVE: p *= maskT (mask pre-transposed + bf16-cast on host).
  - mm2: out[t_tile, 65] = pT.T @ [k | ones]; column 64 accumulates the
    softmax denominator. Two batches packed per PSUM bank.
  - DVE reciprocal+mult normalizes out of PSUM into the output tile.
"""

import os
import numpy as np
import ml_dtypes

import concourse.bass as bass
import concourse.mybir as mybir
import concourse.tile as tile
from concourse.bass_utils import run_bass_kernel_spmd

BF16 = ml_dtypes.bfloat16

T = 1024
S = 1024
B = 8
NH = 8
HD = 64
N_CORES = 8
SCALE = 1.0 / 8.0  # 1/sqrt(hd)

# HAM keep-warm filler counts (measured: no benefit on this instance, whose
# PE appears pinned at 1.2 GHz; kept as knobs)
N_WARM_PRE = int(os.environ.get("N_WARM_PRE", "0"))
N_WARM_ST = int(os.environ.get("N_WARM_ST", "0"))
N_WARM_MM2 = int(os.environ.get("N_WARM_MM2", "0"))


# Empirical per-instruction sem-wait limit for this walrus build: even a
# Matmult with 2 waits fails codegen ("Too many sync wait commands"), so
# every instruction keeps at most one inline wait.
_WAIT_LIMITS = {}


def _split_excess_waits(nc, default_max=1):
    """Hoist excess sem waits off instructions onto standalone
    EventSemaphore waits placed just before them on the same engine queue —
    semantically identical, since each engine executes its queue in order."""
    n = 0
    for f in nc.m.functions:
        for bb in f.blocks:
            insts = bb.instructions
            out = []
            changed = False
            for ins in insts:
                si = ins.sync_info
                waits = list(si.on_wait) if si is not None and si.on_wait else []
                max_waits = _WAIT_LIMITS.get(type(ins).__name__, default_max)
                if (
                    len(waits) > max_waits
                    and type(ins).__name__ != "InstEventSemaphore"
                ):
                    changed = True
                    for w in waits[:-max_waits]:
                        n += 1
                        we = mybir.InstEventSemaphore(
                            name=f"WSPLIT-{n}", ins=[], outs=[]
                        )
                        we.engine = ins.engine
                        we.sync_info = mybir.SyncInfo(on_wait=[w], on_update=[])
                        nc.register_instruction(we)
                        out.append(we)
                    ins.sync_info = mybir.SyncInfo(
                        on_wait=waits[-max_waits:],
                        on_update=list(si.on_update) if si.on_update else [],
                    )
                out.append(ins)
            if changed:
                bb.instructions = out


def build_nc():
    fp32 = mybir.dt.float32
    bf16 = mybir.dt.bfloat16

    nc = bass.Bass(target_bir_lowering=False)
    # Per-core inputs (host pre-sliced/cast/transposed; SPMD: same program,
    # per-core data). qt/kt rows are (b, h) pairs: rows 128p..128p+127 hold
    # batches 2p (partitions 0-63) and 2p+1 (partitions 64-127).
    qt_in = nc.dram_tensor("qt", [B * HD, T], bf16, kind="ExternalInput")
    kt_in = nc.dram_tensor("kt", [B * HD, S], bf16, kind="ExternalInput")
    knat = nc.dram_tensor("knat", [S, B * HD], bf16, kind="ExternalInput")
    maskt = nc.dram_tensor("maskt", [S, T], bf16, kind="ExternalInput")
    out = nc.dram_tensor("out", [T, B * HD], fp32, kind="ExternalOutput")

    knat3 = knat.rearrange("(st p) (b h) -> st p b h", p=128, b=B)

    with tile.TileContext(nc) as tc:
        with (
            tc.tile_pool(name="consts", bufs=1) as consts,
            tc.tile_pool(name="ptp", bufs=6) as ptp,
            tc.tile_pool(name="pte", bufs=3) as pte,
            tc.tile_pool(name="rcp", bufs=4) as rcp,
            tc.tile_pool(name="scp", bufs=2, space="PSUM") as scp,
            tc.tile_pool(name="opp", bufs=1, space="PSUM") as opp,
            tc.tile_pool(name="wmp", bufs=1, space="PSUM") as wmp,
        ):
            # warm the ACT exp table during the DMA preamble
            wsrc = consts.tile([128, 1], mybir.dt.float32, tag="wsrc", name="wsrc")
            wdst = consts.tile([128, 1], bf16, tag="wdst", name="wdst")
            nc.vector.memset(wsrc[:], 0.0)
            nc.scalar.activation(
                wdst[:], wsrc[:], mybir.ActivationFunctionType.Exp
            )
            # --- resident tiles, DMA'd in consumption order ----------------
            # pair-0 q/k first so mm1 starts immediately; mask/k-nat tiles
            # interleaved per s_tile; remaining pairs later.
            qt = [consts.tile([128, T], bf16, tag=f"qt{p}", name=f"qt{p}") for p in range(4)]
            kt = [consts.tile([128, S], bf16, tag=f"kt{p}", name=f"kt{p}") for p in range(4)]
            mt = [consts.tile([128, T], bf16, tag=f"mt{s}", name=f"mt{s}") for s in range(8)]
            kn = [
                consts.tile([128, B, HD + 1], bf16, tag=f"kn{s}", name=f"kn{s}")
                for s in range(8)
            ]
            outt = [
                consts.tile([128, B, HD], fp32, tag=f"out{t}", name=f"out{t}")
                for t in range(8)
            ]

            nc.sync.dma_start(out=qt[0][:], in_=qt_in[0:128, :])
            nc.sync.dma_start(out=kt[0][:], in_=kt_in[0:128, :])
            for st in range(8):
                nc.sync.dma_start(
                    out=mt[st][:], in_=maskt[st * 128 : (st + 1) * 128, :]
                )
                nc.vector.memset(kn[st][:, :, HD], 1.0)
                nc.sync.dma_start(out=kn[st][:, :, 0:HD], in_=knat3[st])
            for p in range(1, 4):
                nc.sync.dma_start(out=qt[p][:], in_=qt_in[p * 128 : (p + 1) * 128, :])
                nc.sync.dma_start(out=kt[p][:], in_=kt_in[p * 128 : (p + 1) * 128, :])

            # HAM keep-warm: the PE clock gate re-throttles to 1.2 GHz
            # whenever the array idles for a window; our PE waits on ACT/DVE
            # between s_tiles, which keeps every matmul at half clock.
            # Discardable filler matmuls (zero-data, rows 0-63 only) during
            # those waits keep the array streaming at 2.4 GHz, which halves
            # the REAL matmul time.
            warm_ps = wmp.tile([128, 512], mybir.dt.float32, tag="warm", name="warm_ps")
            warm_w = consts.tile([64, 128], bf16, tag="warm_w", name="warm_w")
            warm_x = consts.tile([64, 512], bf16, tag="warm_x", name="warm_x")
            nc.vector.memset(warm_w[:], 0.0)
            nc.vector.memset(warm_x[:], 0.0)

            def emit_warm(n):
                for _ in range(n):
                    nc.tensor.matmul(
                        warm_ps[:],
                        warm_w[:],
                        warm_x[:],
                        start=True,
                        stop=True,
                        tile_position=(0, 0),
                    )

            emit_warm(N_WARM_PRE)

            # --- main loop over batch pairs -------------------------------
            # mm2 accumulates into 3 persistent PSUM bank-tiles per pair
            # (8 tt x 2 batches of 65-wide blocks; 3 tt-groups per bank).
            for pair in range(4):
                ops = [
                    opp.tile([128, 512], fp32, tag=f"op{j}", name=f"op{j}_{pair}")
                    for j in range(3)
                ]
                def emit_mm2(st, ptb):
                    # mm2 contributions of s_tile `st` for every t_tile.
                    # start=True clears the WHOLE PSUM bank, so only the
                    # chronologically first matmul into each op tile (per
                    # pair) may carry it; later blocks in the same bank
                    # initialize via per-element has_written bits.
                    for tt in range(8):
                        j, loc = tt // 3, tt % 3
                        for b01 in range(2):
                            b = pair * 2 + b01
                            nc.tensor.matmul(
                                ops[j][
                                    :,
                                    loc * 130 + b01 * 65 : loc * 130 + (b01 + 1) * 65,
                                ],
                                ptb[b01][:, tt * 128 : (tt + 1) * 128],
                                kn[st][:, b, :],
                                start=(st == 0 and loc == 0 and b01 == 0),
                                stop=(st == 7),
                                skip_group_check=True,
                            )

                pts = []
                for st in range(8):
                    ptb = []
                    for b01 in range(2):
                        emit_warm(N_WARM_ST)
                        sc = scp.tile(
                            [128, 1024], fp32, tag="sc", name=f"sc_{pair}_{st}_{b01}"
                        )
                        lhsT = kt[pair][
                            b01 * 64 : (b01 + 1) * 64, st * 128 : (st + 1) * 128
                        ]
                        for th in range(2):
                            rhs = qt[pair][
                                b01 * 64 : (b01 + 1) * 64, th * 512 : (th + 1) * 512
                            ]
                            nc.tensor.matmul(
                                sc[:, th * 512 : (th + 1) * 512],
                                lhsT,
                                rhs,
                                start=True,
                                stop=True,
                                tile_position=(b01 * 64, 0),
                            )
                        pe = pte.tile(
                            [128, 1024], bf16, tag="pe", name=f"pe_{pair}_{st}_{b01}"
                        )
                        nc.scalar.activation(
                            pe[:], sc[:], mybir.ActivationFunctionType.Exp, scale=SCALE
                        )
                        pt = ptp.tile(
                            [128, 1024], bf16, tag="pt", name=f"pt_{pair}_{st}_{b01}"
                        )
                        nc.vector.tensor_tensor(
                            out=pt[:], in0=pe[:], in1=mt[st][:],
                            op=mybir.AluOpType.mult,
                        )
                        ptb.append(pt)
                    pts.append(ptb)
                    emit_warm(N_WARM_MM2)
                    emit_mm2(st, ptb)

                for tt in range(8):
                    j, loc = tt // 3, tt % 3
                    op3 = ops[j][:, loc * 130 : (loc + 1) * 130].rearrange(
                        "p (b x) -> p b x", b=2
                    )
                    rc = rcp.tile([128, 2, 1], fp32, tag="rc", name=f"rc_{pair}_{tt}")
                    nc.vector.reciprocal(rc[:, :, 0], op3[:, :, HD])
                    nc.vector.tensor_tensor(
                        out=outt[tt][:, pair * 2 : (pair + 1) * 2, :],
                        in0=op3[:, :, 0:HD],
                        in1=rc[:].to_broadcast([128, 2, HD]),
                        op=mybir.AluOpType.mult,
                    )

            for tt in range(8):
                nc.sync.dma_start(
                    out=out[tt * 128 : (tt + 1) * 128, :],
                    in_=outt[tt][:],
                )

    _split_excess_waits(nc)
    return nc


_NC_CACHE = None


def _get_nc():
    global _NC_CACHE
    if _NC_CACHE is None:
        _NC_CACHE = build_nc()
    return _NC_CACHE


def kernel(queries: np.ndarray, keys: np.ndarray, attn_mask: np.ndarray) -> np.ndarray:
    assert queries.shape == (T, B, NH * HD)
    assert keys.shape == (S, B, NH * HD)
    assert attn_mask.shape == (B, T, S)

    q_bf = np.asarray(queries, np.float32).astype(BF16)  # [T, B, A]
    k_bf = np.asarray(keys, np.float32).astype(BF16)
    m_bf = np.asarray(attn_mask).astype(BF16)  # bool -> 0.0/1.0

    in_maps = []
    for c in range(N_CORES):
        qs = q_bf[:, :, c * HD : (c + 1) * HD].reshape(T, B * HD)  # [T,(b,h)]
        ks = k_bf[:, :, c * HD : (c + 1) * HD].reshape(S, B * HD)
        in_maps.append(
            {
                "qt": np.ascontiguousarray(qs.T),
                "kt": np.ascontiguousarray(ks.T),
                "knat": np.ascontiguousarray(ks),
                "maskt": np.ascontiguousarray(m_bf[c].T),
            }
        )

    nc = _get_nc()
    res = run_bass_kernel_spmd(nc, in_maps, core_ids=list(range(N_CORES)))
    kernel.last_results = res

    outp = np.empty((T, B, NH * HD), np.float32)
    for c in range(N_CORES):
        outp[:, :, c * HD : (c + 1) * HD] = res.results[c]["out"].reshape(T, B, HD)
    return outp



# revision 11
# speedup vs baseline: 1.0218x; 1.0218x over previous
"""Multi-head attention kernel for Trainium2, 8 NeuronCores.

Problem (NHEAD=8, T=S=1024, B=8, A=512, hd=64):
  q = queries.reshape(T, B*NH, hd); k = keys.reshape(S, B*NH, hd)
  w = softmax(mask(q @ k^T / sqrt(hd)))      per n = b*NH + h, mask = attn_mask[n % NH]
  out = (w @ k).reshape(T, B, A)             (keys double as values)

Sharding: head-parallel. Core c owns head h=c for all 8 batches; every
problem on core c uses the single mask slice attn_mask[c] (n % 8 == h).

Per-core dataflow (bf16 matmuls, f32 PSUM; PE pinned at 1.2 GHz):
  One problem (batch) b at a time, 4 rounds of two s-tiles each. The two
  mm1 matmuls of a round target disjoint PE row groups (tile_position
  (0,0) / (64,0), K=64 each, q rows duplicated into partitions 64-127 on
  host) so they stream CONCURRENTLY - mm1 takes ~1024 not ~2048 col
  cycles per round. Three rotating 2-bank PSUM score buffers decouple
  mm1 from the exp that drains them (2 buffers provably re-serialize
  the pair through ACT). mm2 for round n is emitted after mm1 of round
  n+1 so the PE never waits on the exp->mask latency.
  mm2 accumulates [t, hd|denom] per problem in a 2-bank accumulator:
  tt 0-6 at col tt*65, tt 7 at col 512 (a 65-wide block may not cross
  the 2KB PSUM bank boundary). Output normalized to bf16 and DMA'd out
  per problem, so writeback overlaps compute for the whole run.
"""

import os
import numpy as np
import ml_dtypes

import concourse.bass as bass
import concourse.mybir as mybir
import concourse.tile as tile
from concourse.bass_utils import run_bass_kernel_spmd

BF16 = ml_dtypes.bfloat16

T = 1024
S = 1024
B = 8
NH = 8
HD = 64
N_CORES = 8
SCALE = 1.0 / 8.0  # 1/sqrt(hd)
MM1_N = int(os.environ.get("MM1_N", "512"))  # mm1 moving width (512; 1024 fails walrus ISA check)


# Empirical per-instruction sem-wait limit for this walrus build: even a
# Matmult with 2 waits fails codegen ("Too many sync wait commands"), so
# every instruction keeps at most one inline wait.
def _split_excess_waits(nc, default_max=1):
    """Hoist excess sem waits off instructions onto standalone
    EventSemaphore waits placed just before them on the same engine queue -
    semantically identical, since each engine executes its queue in order."""
    n = 0
    for f in nc.m.functions:
        for bb in f.blocks:
            insts = bb.instructions
            out = []
            changed = False
            for ins in insts:
                si = ins.sync_info
                waits = list(si.on_wait) if si is not None and si.on_wait else []
                if len(waits) > default_max and type(ins).__name__ != "InstEventSemaphore":
                    changed = True
                    for w in waits[:-default_max]:
                        n += 1
                        we = mybir.InstEventSemaphore(
                            name=f"WSPLIT-{n}", ins=[], outs=[]
                        )
                        we.engine = ins.engine
                        we.sync_info = mybir.SyncInfo(on_wait=[w], on_update=[])
                        nc.register_instruction(we)
                        out.append(we)
                    ins.sync_info = mybir.SyncInfo(
                        on_wait=waits[-default_max:],
                        on_update=list(si.on_update) if si.on_update else [],
                    )
                out.append(ins)
            if changed:
                bb.instructions = out


def build_nc():
    fp32 = mybir.dt.float32
    bf16 = mybir.dt.bfloat16

    nc = bass.Bass(target_bir_lowering=False)
    # Host pre-slices per core and duplicates q/k head rows into both PE
    # row-group halves: qkt row-block b holds q_b^T in partitions 0-63 AND
    # 64-127 (so tile_position (64,0) matmuls read partitions 64-127).
    qt_in = nc.dram_tensor("qt", [B * 128, T], bf16, kind="ExternalInput")
    kt_in = nc.dram_tensor("kt", [B * 128, S], bf16, kind="ExternalInput")
    knat = nc.dram_tensor("knat", [S, B * HD], bf16, kind="ExternalInput")
    maskt = nc.dram_tensor("maskt", [S, T], bf16, kind="ExternalInput")
    out = nc.dram_tensor("out", [T, B * HD], bf16, kind="ExternalOutput")

    knat3 = knat.rearrange("(st p) (b h) -> st p b h", p=128, b=B)
    out3 = out.rearrange("(tt p) (b h) -> b p tt h", p=128, b=B)

    with tile.TileContext(nc) as tc:
        with (
            tc.tile_pool(name="consts", bufs=1) as consts,
            tc.tile_pool(name="ptp", bufs=6) as ptp,
            tc.tile_pool(name="pte", bufs=4) as pte,
            tc.tile_pool(name="rcp", bufs=2) as rcp,
            tc.tile_pool(name="otp", bufs=2) as otp,
            tc.tile_pool(name="scp", bufs=3, space="PSUM") as scp,
            tc.tile_pool(name="opp", bufs=1, space="PSUM") as opp,
        ):
            # warm the ACT exp table during the DMA preamble
            wsrc = consts.tile([128, 1], fp32, tag="wsrc", name="wsrc")
            wdst = consts.tile([128, 1], bf16, tag="wdst", name="wdst")
            nc.vector.memset(wsrc[:], 0.0)
            nc.scalar.activation(wdst[:], wsrc[:], mybir.ActivationFunctionType.Exp)

            # --- resident tiles, DMA'd in consumption order ----------------
            qt = [consts.tile([128, T], bf16, tag=f"qt{b}", name=f"qt{b}") for b in range(B)]
            kt = [consts.tile([128, S], bf16, tag=f"kt{b}", name=f"kt{b}") for b in range(B)]
            mt = [consts.tile([128, T], bf16, tag=f"mt{s}", name=f"mt{s}") for s in range(8)]
            kn = [
                consts.tile([128, B, HD + 1], bf16, tag=f"kn{s}", name=f"kn{s}")
                for s in range(8)
            ]

            # problem 0's q/k first so mm1 starts immediately, then mask and
            # k-nat tiles in s-tile order, then the remaining problems' q/k.
            nc.sync.dma_start(out=qt[0][:], in_=qt_in[0:128, :])
            nc.sync.dma_start(out=kt[0][:], in_=kt_in[0:128, :])
            for st in range(8):
                nc.sync.dma_start(out=mt[st][:], in_=maskt[st * 128 : (st + 1) * 128, :])
                nc.vector.memset(kn[st][:, :, HD], 1.0)
                nc.sync.dma_start(out=kn[st][:, :, 0:HD], in_=knat3[st])
            for b in range(1, B):
                nc.sync.dma_start(out=qt[b][:], in_=qt_in[b * 128 : (b + 1) * 128, :])
                nc.sync.dma_start(out=kt[b][:], in_=kt_in[b * 128 : (b + 1) * 128, :])

            # mm2 accumulator layout inside a [128, 1024] (2-bank) tile:
            # tt 0-6 -> 65-wide blocks at tt*65 (<=455+65=520... block 6 ends
            # at 454; all within bank 0 plus into bank 1? no: 6*65+65=455).
            # Columns: tt*65 for tt in 0..6 occupy 0..454 (bank 0 is 512 f32),
            # tt 7 at 512 starts bank 1. Nothing crosses a bank boundary.
            OFF = [tt * 65 for tt in range(7)] + [512]

            def emit_mm1(b, r):
                """Two concurrent mm1 matmuls for s-tiles 2r, 2r+1 into
                rotating sc tiles (distinct PE row groups)."""
                # Interleave the two row-group streams chunk by chunk
                # (A0,B0,A1,B1): matmuls execute in order, and only adjacent
                # matmuls with disjoint row groups run concurrently.
                scs = [
                    (
                        2 * r + half,
                        scp.tile(
                            [128, 1024], fp32, tag="sc", name=f"sc_{b}_{2*r+half}"
                        ),
                    )
                    for half in range(2)
                ]
                for i in range(0, 1024, MM1_N):
                    for half, (st, sc) in enumerate(scs):
                        lo = half * 64
                        nc.tensor.matmul(
                            sc[:, i : i + MM1_N],
                            kt[b][lo : lo + 64, st * 128 : (st + 1) * 128],
                            qt[b][lo : lo + 64, i : i + MM1_N],
                            start=True,
                            stop=True,
                            tile_position=(lo, 0),
                        )
                return scs

            def emit_exp_mask(b, scs):
                pts = []
                for st, sc in scs:
                    pe = pte.tile([128, 1024], bf16, tag="pe", name=f"pe_{b}_{st}")
                    nc.scalar.activation(
                        pe[:], sc[:], mybir.ActivationFunctionType.Exp, scale=SCALE
                    )
                    pt = ptp.tile([128, 1024], bf16, tag="pt", name=f"pt_{b}_{st}")
                    nc.vector.tensor_tensor(
                        out=pt[:], in0=pe[:], in1=mt[st][:], op=mybir.AluOpType.mult
                    )
                    pts.append((st, pt))
                return pts

            def emit_mm2(b, ops, pts, first):
                for st, pt in pts:
                    for tt in range(8):
                        nc.tensor.matmul(
                            ops[:, OFF[tt] : OFF[tt] + 65],
                            pt[:, tt * 128 : (tt + 1) * 128],
                            kn[st][:, b, :],
                            start=(first and st % 2 == 0 and tt in (0, 7)),
                            stop=(st == 7),
                            skip_group_check=True,
                        )

            def emit_norm(b, ops):
                rc = rcp.tile([128, 8, 1], fp32, tag="rc", name=f"rc_{b}")
                ops7 = ops[:, 0 : 7 * 65].rearrange("p (tt x) -> p tt x", x=65)
                nc.vector.reciprocal(rc[:, 0:7, 0], ops7[:, :, HD])
                nc.vector.reciprocal(rc[:, 7, 0:1], ops[:, 512 + HD : 512 + HD + 1])
                ot = otp.tile([128, 8, HD], bf16, tag="ot", name=f"ot_{b}")
                nc.vector.tensor_tensor(
                    out=ot[:, 0:7, :],
                    in0=ops7[:, :, 0:HD],
                    in1=rc[:, 0:7, :].to_broadcast([128, 7, HD]),
                    op=mybir.AluOpType.mult,
                )
                nc.vector.tensor_tensor(
                    out=ot[:, 7, :],
                    in0=ops[:, 512 : 512 + HD],
                    in1=rc[:, 7, :].to_broadcast([128, HD]),
                    op=mybir.AluOpType.mult,
                )
                nc.sync.dma_start(out=out3[b], in_=ot[:])

            # --- main loop: 32 rounds, mm2 trails mm1 by one round. The
            # accumulator for problem pb is allocated when its first mm2 is
            # emitted (after problem pb-1's norm), so the single-buffer pool
            # sees a clean write-after-read ordering.
            prev = None  # (pb, pr, ppts)
            ops_cur = None
            for n in range(32):
                b, r = divmod(n, 4)
                scs = emit_mm1(b, r)
                pts = emit_exp_mask(b, scs)
                if prev is not None:
                    pb, pr, ppts = prev
                    if pr == 0:
                        ops_cur = opp.tile(
                            [128, 1024], fp32, tag="ops", name=f"ops_{pb}"
                        )
                    emit_mm2(pb, ops_cur, ppts, first=(pr == 0))
                    if pr == 3:
                        emit_norm(pb, ops_cur)
                prev = (b, r, pts)
            pb, pr, ppts = prev
            emit_mm2(pb, ops_cur, ppts, first=False)
            emit_norm(pb, ops_cur)

    _split_excess_waits(nc)
    return nc


_NC_CACHE = None


def _get_nc():
    global _NC_CACHE
    if _NC_CACHE is None:
        _NC_CACHE = build_nc()
    return _NC_CACHE


def kernel(queries: np.ndarray, keys: np.ndarray, attn_mask: np.ndarray) -> np.ndarray:
    assert queries.shape == (T, B, NH * HD)
    assert keys.shape == (S, B, NH * HD)
    assert attn_mask.shape == (B, T, S)

    q_bf = np.asarray(queries, np.float32).astype(BF16)  # [T, B, A]
    k_bf = np.asarray(keys, np.float32).astype(BF16)
    m_bf = np.asarray(attn_mask).astype(BF16)  # bool -> 0.0/1.0

    in_maps = []
    for c in range(N_CORES):
        qs = q_bf[:, :, c * HD : (c + 1) * HD]  # [T, B, 64]
        ks = k_bf[:, :, c * HD : (c + 1) * HD]
        # [B, 128, T]: per problem b, q_b^T duplicated into both row halves
        qt2 = np.empty((B, 128, T), BF16)
        kt2 = np.empty((B, 128, S), BF16)
        for b in range(B):
            qT = np.ascontiguousarray(qs[:, b, :].T)
            kT = np.ascontiguousarray(ks[:, b, :].T)
            qt2[b, 0:64] = qT
            qt2[b, 64:128] = qT
            kt2[b, 0:64] = kT
            kt2[b, 64:128] = kT
        in_maps.append(
            {
                "qt": qt2.reshape(B * 128, T),
                "kt": kt2.reshape(B * 128, S),
                "knat": np.ascontiguousarray(ks.reshape(S, B * HD)),
                "maskt": np.ascontiguousarray(m_bf[c].T),
            }
        )

    nc = _get_nc()
    res = run_bass_kernel_spmd(nc, in_maps, core_ids=list(range(N_CORES)))
    kernel.last_results = res

    outp = np.empty((T, B, NH * HD), np.float32)
    for c in range(N_CORES):
        outp[:, :, c * HD : (c + 1) * HD] = (
            res.results[c]["out"].astype(np.float32).reshape(T, B, HD)
        )
    return outp
